# revision 1
# baseline (speedup 1.0000x reference)
"""Trainium2 Bass kernel for the DFL-FCOS detection head (nn_DFLFCOS_10909216932636).

Sharding: 8 cores = 4 images x 2 spatial halves (H split). Each core runs an
identical SPMD program over 6 "tiles": level-0 is split into two W-halves
(t0/t1) so activations fit SBUF; levels 1-4 are t2..t5. Conv3x3 is computed as
9 shifted 1x1 convs accumulated in PSUM via the tensor engine (fp32 storage,
float32r matmul mode). Halos of 5 rows/cols cover the 5-deep conv receptive
field; image-edge zero padding is reproduced exactly by shift clipping at
slab edges (tower biases are zero, so relu(conv(0))=0 matches zero padding).

Outputs per core: cls logits [80, P] and DFL-decoded boxes [4, P] over each
tile's full slab grid; the host slices the valid region and reassembles the
full [4, 20267, 84] output.
"""

import numpy as np

REG_MAX = 8
NUM_CLASSES = 80
C = 256
NUM_CONVS = 4
LEVEL_SHAPES = [(100, 152), (50, 76), (25, 38), (13, 19), (7, 10)]
B = 4
N_CORES = 8

F32 = np.float32


class T:
    def __init__(self, name, level, Hs, Wc, Wv, cstart, goff, ooff):
        self.name = name      # tensor name of the per-core input slab
        self.level = level
        self.Hs = Hs          # slab rows (Hh + 5)
        self.Wc = Wc          # conv output width (even, fp32r requirement)
        self.Ws = Wc + 2      # buffer row stride: [lpad][Wc data cols][rpad]
        self.Wv = Wv          # valid width (real data cols in slab)
        self.cstart = cstart  # image col of data col 0
        self.goff = goff      # column offset of this tile inside its group buffer
        self.ooff = ooff      # column offset into out_cls / out_dec


def _mk_tiles():
    # (level, Hs, Wc_alloc, Wv, cstart)
    cfg = [
        ("x0", 0, 55, 82, 81, 0),
        ("x1", 0, 55, 82, 81, 71),
        ("x2", 1, 30, 76, 76, 0),
        ("x3", 2, 18, 38, 38, 0),
        ("x4", 3, 12, 24, 19, 0),
        ("x5", 4, 7, 44, 10, 0),
    ]
    tiles = []
    ooff = 0
    for name, lv, Hs, Wc, Wv, cs in cfg:
        tiles.append(T(name, lv, Hs, Wc, Wv, cs, 0, ooff))
        ooff += Hs * Wc
    # group buffer offsets: groups are [t0], [t1], [t2,t3,t4,t5]
    go = 0
    for t in tiles[2:]:
        t.goff = go
        go += 2 * t.Hs * t.Ws
    assert go <= 2 * 55 * 84, go
    return tiles, ooff


TILES, OUT_P = _mk_tiles()
GROUPS = [[TILES[0]], [TILES[1]], TILES[2:]]
ACT_COLS = 2 * 55 * 84  # flat activation buffer columns (2 ci blocks)
HH = [50, 25, 13, 7, 4]  # ceil(H/2) per level


def _chunks(t):
    hmax = 512 // t.Wc
    nch = -(-t.Hs // hmax)
    base, rem = divmod(t.Hs, nch)
    out = []
    r0 = 0
    for i in range(nch):
        h = base + (1 if i < rem else 0)
        out.append((r0, h))
        r0 += h
    return out


def _clip(r0, h, d, size):
    """Output rows [r0, r0+h) with shift d in {0,1,2}; input row = out + d - 1.
    Returns (lo, n, in0): lo rows dropped at start, n rows kept, in0 input row."""
    lo = max(r0, 1 - d) - r0
    hi = min(r0 + h, size + 1 - d) - r0
    return lo, hi - lo, r0 + lo + d - 1


def build_nc():
    import concourse.bass as bass  # noqa: F401
    import concourse.mybir as mybir
    import concourse.tile as tile
    from concourse import bacc

    f32 = mybir.dt.float32
    f32r = mybir.dt.float32r
    AF = mybir.ActivationFunctionType

    nc = bacc.Bacc(None, target_bir_lowering=False, debug=False, enable_asserts=False)

    xin = {
        t.name: nc.dram_tensor(t.name, [C, t.Hs, t.Ws], f32r, kind="ExternalInput")
        for t in TILES
    }
    ctw = nc.dram_tensor("ctw", [NUM_CONVS, 128, 36, 128], f32r, kind="ExternalInput")
    btw = nc.dram_tensor("btw", [NUM_CONVS, 128, 36, 128], f32r, kind="ExternalInput")
    chw = nc.dram_tensor("chw", [128, 18, NUM_CLASSES], f32r, kind="ExternalInput")
    bhw = nc.dram_tensor("bhw", [128, 18, 36], f32r, kind="ExternalInput")
    bias = nc.dram_tensor("bias", [128, 18], f32, kind="ExternalInput")
    dfl = nc.dram_tensor("dfl", [36, 36], f32r, kind="ExternalInput")
    zc = nc.dram_tensor("zc", [128, 55], f32r, kind="ExternalInput")
    out_cls = nc.dram_tensor("out_cls", [NUM_CLASSES, OUT_P], f32, kind="ExternalOutput")
    out_dec = nc.dram_tensor("out_dec", [4, OUT_P], f32, kind="ExternalOutput")

    with tile.TileContext(nc) as tc:
        with (
            tc.tile_pool(name="px", bufs=1) as px,
            tc.tile_pool(name="pb", bufs=1) as pb,
            tc.tile_pool(name="pc", bufs=1) as pc,
            tc.tile_pool(name="pw", bufs=2) as pw,
            tc.tile_pool(name="pconst", bufs=1) as pconst,
            tc.tile_pool(name="pstage", bufs=3) as pstage,
            tc.tile_pool(name="psA", bufs=4, space="PSUM") as psA,
            tc.tile_pool(name="psB", bufs=2, space="PSUM") as psB,
            tc.tile_pool(name="psD", bufs=2, space="PSUM") as psD,
        ):
            # constants loaded once
            whc = pconst.tile([128, 18, NUM_CLASSES], f32r, tag="whc")
            nc.sync.dma_start(whc[:, :, :], chw[:, :, :])
            whb = pconst.tile([128, 18, 36], f32r, tag="whb")
            nc.sync.dma_start(whb[:, :, :], bhw[:, :, :])
            bt = pconst.tile([128, 18], f32, tag="bt")
            nc.sync.dma_start(bt[:, :], bias[:, :])
            dflt = pconst.tile([36, 36], f32r, tag="dflt")
            nc.sync.dma_start(dflt[:, :], dfl[:, :])

            def av(buf, t, cib, r0, h, c0, w):
                base = t.goff + cib * t.Hs * t.Ws
                v = buf[:, base : base + t.Hs * t.Ws].rearrange(
                    "p (h w) -> p h w", w=t.Ws
                )
                return v[:, r0 : r0 + h, c0 : c0 + w]

            def conv_tower(tiles, src, dst, wt, bias_base):
                for t in tiles:
                    for r0, h in _chunks(t):
                        for cob in range(2):
                            ps = psA.tile([128, h, t.Wc], f32, tag="pca", name="ps")
                            idx = 0
                            for cib in range(2):
                                for dy in range(3):
                                    for dx in range(3):
                                        rlo, rn, rin = _clip(r0, h, dy, t.Hs)
                                        tf = ((cib * 3 + dy) * 3 + dx) * 2 + cob
                                        nc.tensor.matmul(
                                            ps[:, rlo : rlo + rn, 0 : t.Wc],
                                            wt[:, tf, :],
                                            av(src, t, cib, rin, rn, dx, t.Wc),
                                            start=(idx == 0),
                                            stop=(idx == 17),
                                            skip_group_check=True,
                                        )
                                        idx += 1
                            nc.scalar.activation(
                                av(dst, t, cob, r0, h, 1, t.Wc),
                                ps[:, :, :],
                                AF.Relu,
                                bias=bt[:, bias_base + cob : bias_base + cob + 1],
                            )
                    if t.Wv < t.Wc:
                        # re-zero buffer col Wv+1 (= out col Wv) so the next
                        # conv's dx=2 read at out col Wv-1 sees right-edge zeros
                        for cib in range(2):
                            nc.sync.dma_start(
                                av(dst, t, cib, 0, t.Hs, t.Wv + 1, 1), zc[:, 0 : t.Hs]
                            )

            def head_cls(tiles, src):
                for t in tiles:
                    for r0, h in _chunks(t):
                        n = h * t.Wc
                        ps = psB.tile([NUM_CLASSES, h, t.Wc], f32, tag="psb", name="ps")
                        idx = 0
                        for cib in range(2):
                            for dy in range(3):
                                for dx in range(3):
                                    rlo, rn, rin = _clip(r0, h, dy, t.Hs)
                                    tf = (cib * 3 + dy) * 3 + dx
                                    nc.tensor.matmul(
                                        ps[:, rlo : rlo + rn, 0 : t.Wc],
                                        whc[:, tf, :],
                                        av(src, t, cib, rin, rn, dx, t.Wc),
                                        start=(idx == 0),
                                        stop=(idx == 17),
                                        skip_group_check=True,
                                    )
                                    idx += 1
                        st = pstage.tile([NUM_CLASSES, 512], f32, tag="st", name="st")
                        nc.scalar.activation(
                            st[:, 0:n], ps[:, :, :], AF.Identity, bias=bt[0:NUM_CLASSES, 16:17]
                        )
                        off = t.ooff + r0 * t.Wc
                        nc.sync.dma_start(out_cls[:, off : off + n], st[:, 0:n])

            def head_box(tiles, src):
                for t in tiles:
                    for r0, h in _chunks(t):
                        n = h * t.Wc
                        ps = psB.tile([36, h, t.Wc], f32, tag="psb", name="ps")
                        idx = 0
                        for cib in range(2):
                            for dy in range(3):
                                for dx in range(3):
                                    rlo, rn, rin = _clip(r0, h, dy, t.Hs)
                                    tf = (cib * 3 + dy) * 3 + dx
                                    nc.tensor.matmul(
                                        ps[:, rlo : rlo + rn, 0 : t.Wc],
                                        whb[:, tf, :],
                                        av(src, t, cib, rin, rn, dx, t.Wc),
                                        start=(idx == 0),
                                        stop=(idx == 17),
                                        skip_group_check=True,
                                    )
                                    idx += 1
                        ex = pstage.tile([36, 512], f32r, tag="ex", name="ex")
                        nc.scalar.activation(
                            ex[:, 0:n], ps[:, :, :], AF.Exp, bias=bt[0:36, 17:18]
                        )
                        pd = psD.tile([36, 512], f32, tag="psd", name="pd")
                        nc.tensor.matmul(
                            pd[:, 0:n],
                            dflt[:, :],
                            ex[:, 0:n],
                            start=True,
                            stop=True,
                        )
                        rc = pstage.tile([4, 512], f32, tag="rc", name="rc")
                        nc.vector.reciprocal(rc[:, 0:n], pd[32:36, 0:n])
                        dc = pstage.tile([4, 512], f32, tag="dc", name="dc")
                        nc.vector.tensor_mul(dc[:, 0:n], pd[0:4, 0:n], rc[:, 0:n])
                        off = t.ooff + r0 * t.Wc
                        nc.sync.dma_start(out_dec[:, off : off + n], dc[:, 0:n])

            for tiles in GROUPS:
                xb = px.tile([128, ACT_COLS], f32r, tag="x", name="xb")
                bb = pb.tile([128, ACT_COLS], f32r, tag="b", name="bb")
                cb = pc.tile([128, ACT_COLS], f32r, tag="c", name="cb")
                for t in tiles:
                    for cib in range(2):
                        nc.sync.dma_start(
                            av(xb, t, cib, 0, t.Hs, 0, t.Ws),
                            xin[t.name][cib * 128 : (cib + 1) * 128, :, :],
                        )
                        for buf in (bb, cb):
                            nc.sync.dma_start(
                                av(buf, t, cib, 0, t.Hs, 0, 1), zc[:, 0 : t.Hs]
                            )
                            nc.sync.dma_start(
                                av(buf, t, cib, 0, t.Hs, t.Wc + 1, 1), zc[:, 0 : t.Hs]
                            )
                src = xb
                for k in range(NUM_CONVS):
                    wt = pw.tile([128, 36, 128], f32r, tag="w", name="wt")
                    nc.sync.dma_start(wt[:, :, :], ctw[k, :, :, :])
                    dst = bb if k % 2 == 0 else cb
                    conv_tower(tiles, src, dst, wt, k * 2)
                    src = dst
                head_cls(tiles, src)
                src = xb
                for k in range(NUM_CONVS):
                    wt = pw.tile([128, 36, 128], f32r, tag="w", name="wt")
                    nc.sync.dma_start(wt[:, :, :], btw[k, :, :, :])
                    dst = bb if k % 2 == 0 else cb
                    conv_tower(tiles, src, dst, wt, 8 + k * 2)
                    src = dst
                head_box(tiles, src)

    nc.finalize()
    return nc


def round_f32r(a):
    """Round fp32 to the PE's fp32r format: 11-bit mantissa, RNE, low 12 bits zero."""
    u = np.ascontiguousarray(a, dtype=np.float32).view(np.uint32).copy()
    lsb = (u >> 12) & 1
    u += 0x7FF + lsb
    u &= 0xFFFFF000
    return u.view(np.float32)


def _prep_in_maps(inputs):
    feats = [np.ascontiguousarray(np.asarray(inputs[f"feat{i}"], dtype=F32)) for i in range(5)]
    cls_tw = np.asarray(inputs["cls_tw"], dtype=F32)
    cls_tb = np.asarray(inputs["cls_tb"], dtype=F32)
    box_tw = np.asarray(inputs["box_tw"], dtype=F32)
    box_tb = np.asarray(inputs["box_tb"], dtype=F32)
    cls_w = np.asarray(inputs["cls_w"], dtype=F32)
    cls_b = np.asarray(inputs["cls_b"], dtype=F32)
    box_w = np.asarray(inputs["box_w"], dtype=F32)
    box_b = np.asarray(inputs["box_b"], dtype=F32)

    # tower weights -> [k, ci, (cib,dy,dx,cob), co]
    def prep_tw(w):
        a = w.reshape(NUM_CONVS, 2, 128, 2, 128, 3, 3)  # k,cob,co,cib,ci,dy,dx
        a = a.transpose(0, 4, 3, 5, 6, 1, 2)  # k,ci,cib,dy,dx,cob,co
        return np.ascontiguousarray(a.reshape(NUM_CONVS, 128, 36, 128))

    # head weights -> [ci, (cib,dy,dx), co]
    def prep_hw(w):
        o = w.shape[0]
        a = w.reshape(o, 2, 128, 3, 3)  # co,cib,ci,dy,dx
        a = a.transpose(2, 1, 3, 4, 0)  # ci,cib,dy,dx,co
        return np.ascontiguousarray(a.reshape(128, 18, o))

    ctw = round_f32r(prep_tw(cls_tw))
    btw = round_f32r(prep_tw(box_tw))
    chw = round_f32r(prep_hw(cls_w))
    bhw = round_f32r(prep_hw(box_w))

    bias = np.zeros((128, 18), dtype=F32)
    for k in range(NUM_CONVS):
        for cob in range(2):
            bias[:, k * 2 + cob] = cls_tb[k, cob * 128 : (cob + 1) * 128]
            bias[:, 8 + k * 2 + cob] = box_tb[k, cob * 128 : (cob + 1) * 128]
    bias[0:NUM_CLASSES, 16] = cls_b
    bias[0:36, 17] = box_b

    proj = np.arange(REG_MAX + 1, dtype=F32)
    dfl = np.zeros((36, 36), dtype=F32)
    for k in range(4):
        dfl[9 * k : 9 * k + 9, k] = proj
        dfl[9 * k : 9 * k + 9, 32 + k] = 1.0

    shared = {
        "ctw": ctw, "btw": btw, "chw": chw, "bhw": bhw, "bias": bias, "dfl": dfl,
        "zc": np.zeros((128, 55), dtype=F32),
    }

    in_maps = []
    for c in range(N_CORES):
        b, h = divmod(c, 2)
        m = dict(shared)
        for t in TILES:
            H, W = LEVEL_SHAPES[t.level]
            f = feats[t.level][b]
            rstart = 0 if h == 0 else H - t.Hs
            slab = np.zeros((C, t.Hs, t.Ws), dtype=F32)
            r0 = max(0, -rstart)
            r1 = min(t.Hs, H - rstart)
            c1 = min(t.Wv, W - t.cstart)
            slab[:, r0:r1, 1 : 1 + c1] = f[
                :, rstart + r0 : rstart + r1, t.cstart : t.cstart + c1
            ]
            m[t.name] = round_f32r(slab)
        in_maps.append(m)
    return in_maps


def _assemble(results):
    HW = [h * w for h, w in LEVEL_SHAPES]
    lvl_off = np.cumsum([0] + HW)
    P_total = lvl_off[-1]
    out = np.zeros((B, P_total, NUM_CLASSES + 4), dtype=F32)
    for c in range(N_CORES):
        b, h = divmod(c, 2)
        ocls = results[c]["out_cls"]
        odec = results[c]["out_dec"]
        for t in TILES:
            H, W = LEVEL_SHAPES[t.level]
            Hh = HH[t.level]
            blk_c = ocls[:, t.ooff : t.ooff + t.Hs * t.Wc].reshape(NUM_CLASSES, t.Hs, t.Wc)
            blk_d = odec[:, t.ooff : t.ooff + t.Hs * t.Wc].reshape(4, t.Hs, t.Wc)
            # valid image rows for this half
            y0, y1 = h * Hh, min(H, (h + 1) * Hh)
            rstart = 0 if h == 0 else H - t.Hs
            sr0 = y0 - rstart
            # valid image cols for this tile
            if t.level == 0:
                x0, x1 = (0, 76) if t.cstart == 0 else (76, 152)
            else:
                x0, x1 = 0, W
            sc0 = x0 - t.cstart
            yy = np.arange(y0, y1)
            pos = lvl_off[t.level] + yy[:, None] * W + np.arange(x0, x1)[None, :]
            out[b, pos, 0:NUM_CLASSES] = blk_c[
                :, sr0 : sr0 + (y1 - y0), sc0 : sc0 + (x1 - x0)
            ].transpose(1, 2, 0)
            out[b, pos, NUM_CLASSES:] = blk_d[
                :, sr0 : sr0 + (y1 - y0), sc0 : sc0 + (x1 - x0)
            ].transpose(1, 2, 0)
    return out


_CACHE = {}


def _get_nc():
    if "nc" not in _CACHE:
        _CACHE["nc"] = build_nc()
    return _CACHE["nc"]


def kernel(**inputs):
    from concourse.bass_utils import run_bass_kernel_spmd

    nc = _get_nc()
    in_maps = _prep_in_maps(inputs)
    res = run_bass_kernel_spmd(nc, in_maps, core_ids=list(range(N_CORES)))
    return _assemble(res.results)

